# revision 8
# baseline (speedup 1.0000x reference)
"""Cross-attention fusion kernel for Trainium2 (8 NeuronCores, SPMD).

Computes O = softmax(Q @ K^T) @ V with Q = hidden_states [32,1024,768],
K = V = img_hidden_state [32,576,768], all fp32.

Sharding: data-parallel over batch — 4 batches per core, no collectives.

Per-core dataflow (per batch):
  - V (=K) loaded in natural [m, d] layout; K^T built via PE transposes.
  - Per 128-row Q tile: PE-transpose Q -> Q^T [d, n]; S = Q^T.T @ K^T
    accumulated over d in two PSUM half-banks; softmax along free axis
    (DVE max-reduce, ACT fused exp+row-sum); P^T via PE transposes;
    O = P^T.T @ V accumulated over m; 1/rowsum folded into the
    PSUM->SBUF output copy on ACT.

Matmuls use float32r: fp32 with mantissa rounded to 11 bits, running at
bf16 PE rate (1 cycle/row for free dims >= 256). Inputs are pre-rounded
to the f32r grid on the host so every f32r operand is exact.
"""

from contextlib import ExitStack

import jax
import numpy as np
from jax.experimental.shard_map import shard_map
from jax.sharding import Mesh, PartitionSpec

import concourse.bass as bass
import concourse.tile as tile
from concourse import bass2jax, mybir

F32 = mybir.dt.float32
F32R = mybir.dt.float32r

N_CORES = 8
B, N, M, D = 32, 1024, 576, 768
B_LOC = B // N_CORES  # 4 batches per core
P = 128
NT = N // P  # 8 query tiles per batch
DT = D // P  # 6 contraction tiles
# m tiles: 4 full 128-partition tiles + one 64-row edge tile
M_TILES = [(0, 128), (128, 128), (256, 128), (384, 128), (512, 64)]
MH = 288  # half of M; both S matmul free dims >= 256 keep f32r at full rate


def round_f32r(a: np.ndarray) -> np.ndarray:
    """Round fp32 to the f32r grid (11-bit mantissa, round-half-even)."""
    u = np.ascontiguousarray(a, dtype=np.float32).view(np.uint32)
    low = u & np.uint32(0xFFF)
    base = u & ~np.uint32(0xFFF)
    add = (low > 0x800) | ((low == 0x800) & ((u >> 12) & 1).astype(bool))
    return (base + np.where(add, np.uint32(0x1000), np.uint32(0))).view(np.float32)


def split_multi_waits(nc):
    """Walrus in this toolchain rejects instructions with more than one sync
    wait. Hoist excess waits onto same-engine NoOp carriers placed directly
    before the instruction; engines execute their stream in order, so the
    wait conditions still hold before the instruction issues."""
    carrier_id = 0
    for func in nc.m.functions:
        for bb in func.blocks:
            insts = list(bb.instructions)
            out = []
            changed = False
            for inst in insts:
                si = inst.sync_info
                waits = list(si.on_wait) if si is not None else []
                if len(waits) > 1:
                    changed = True
                    for w in waits[:-1]:
                        nop = mybir.InstNoOp(
                            name=f"waitc-{carrier_id}", engine=inst.engine
                        )
                        carrier_id += 1
                        nop.sync_info = mybir.SyncInfo(on_wait=[w], on_update=[])
                        out.append(nop)
                    inst.sync_info = mybir.SyncInfo(
                        on_wait=waits[-1:], on_update=list(si.on_update)
                    )
                out.append(inst)
            if changed:
                bb.instructions = out


def build_program(b_loc: int = B_LOC, repeat: int = 1):
    nc = bass.Bass("TRN2", target_bir_lowering=False, debug=False)
    hid = nc.dram_tensor("hidden", [b_loc, N, D], F32R, kind="ExternalInput").ap()
    img = nc.dram_tensor("img", [b_loc, M, D], F32R, kind="ExternalInput").ap()
    idn = nc.dram_tensor("ident", [P, P], F32R, kind="ExternalInput").ap()
    out = nc.dram_tensor("out", [b_loc, N, D], F32, kind="ExternalOutput").ap()

    with tile.TileContext(nc) as tc, ExitStack() as ctx:
        rep_cm = tc.For_i(0, repeat, 1) if repeat > 1 else None
        if rep_cm is not None:
            ctx.enter_context(rep_cm)
        const_pool = ctx.enter_context(tc.tile_pool(name="const", bufs=1))
        ident = const_pool.tile([P, P], F32R)
        nc.sync.dma_start(out=ident, in_=idn[:, :])

        kv_pool = ctx.enter_context(tc.tile_pool(name="kv", bufs=2))
        q_pool = ctx.enter_context(tc.tile_pool(name="q", bufs=3))
        qt_pool = ctx.enter_context(tc.tile_pool(name="qt", bufs=3))
        p_pool = ctx.enter_context(tc.tile_pool(name="p", bufs=3))
        pt_pool = ctx.enter_context(tc.tile_pool(name="pt", bufs=4))
        o_pool = ctx.enter_context(tc.tile_pool(name="o", bufs=3))
        stat_pool = ctx.enter_context(tc.tile_pool(name="stat", bufs=4))
        ps_s = ctx.enter_context(tc.tile_pool(name="ps_s", bufs=2, space="PSUM"))
        ps_t = ctx.enter_context(tc.tile_pool(name="ps_t", bufs=2, space="PSUM"))
        ps_o = ctx.enter_context(tc.tile_pool(name="ps_o", bufs=1, space="PSUM"))

        for b in range(b_loc):
            # V in natural [m, d] layout; doubles as the K-transpose source.
            v = kv_pool.tile([P, 5, D], F32R, tag="v")
            for mi, (m0, msz) in enumerate(M_TILES):
                nc.sync.dma_start(out=v[:msz, mi, :], in_=img[b, m0 : m0 + msz, :])
            # K^T [d, m]: six 128-partition d tiles side by side in free dim.
            kT = kv_pool.tile([P, DT, M], F32R, tag="kT")
            for mi, (m0, msz) in enumerate(M_TILES):
                for j in range(DT):
                    tp = ps_t.tile([P, P], F32R, tag="tp")
                    nc.tensor.transpose(
                        tp[:, :msz],
                        v[:msz, mi, j * P : (j + 1) * P],
                        ident[:msz, :msz],
                    )
                    nc.any.tensor_copy(out=kT[:, j, m0 : m0 + msz], in_=tp[:, :msz])

            for nt in range(NT):
                q = q_pool.tile([P, D], F32R, tag="q")
                nc.sync.dma_start(out=q, in_=hid[b, nt * P : (nt + 1) * P, :])
                qT = qt_pool.tile([P, DT, P], F32R, tag="qT")
                for j in range(DT):
                    tp = ps_t.tile([P, P], F32R, tag="tp")
                    nc.tensor.transpose(
                        tp, q[:, j * P : (j + 1) * P], ident
                    )
                    nc.any.tensor_copy(out=qT[:, j, :], in_=tp)

                # S = Q K^T, [128, 576] in two half-bank PSUM tiles.
                s = ps_s.tile([P, 2, 512], F32, tag="s")
                for j in range(DT):
                    qTj = qT[:, j, :]
                    nc.tensor.matmul(
                        s[:, 0, :MH],
                        qTj,
                        kT[:, j, 0:MH],
                        start=(j == 0),
                        stop=(j == DT - 1),
                    )
                    nc.tensor.matmul(
                        s[:, 1, :MH],
                        qTj,
                        kT[:, j, MH:M],
                        start=(j == 0),
                        stop=(j == DT - 1),
                    )

                # Softmax over the free (m) axis.
                nmax = stat_pool.tile([P, 1], F32, tag="nmax")
                nc.vector.tensor_reduce(
                    out=nmax,
                    in_=s[:, :, :MH],
                    axis=mybir.AxisListType.XY,
                    op=mybir.AluOpType.max,
                    negate=True,
                )
                p = p_pool.tile([P, M], F32R, tag="p")
                sum0 = stat_pool.tile([P, 1], F32, tag="sum0")
                sum1 = stat_pool.tile([P, 1], F32, tag="sum1")
                nc.scalar.activation(
                    out=p[:, 0:MH],
                    in_=s[:, 0, :MH],
                    func=mybir.ActivationFunctionType.Exp,
                    bias=nmax,
                    scale=1.0,
                    accum_out=sum0,
                )
                nc.scalar.activation(
                    out=p[:, MH:M],
                    in_=s[:, 1, :MH],
                    func=mybir.ActivationFunctionType.Exp,
                    bias=nmax,
                    scale=1.0,
                    accum_out=sum1,
                )
                rsum = stat_pool.tile([P, 1], F32, tag="rsum")
                nc.vector.tensor_add(out=rsum, in0=sum0, in1=sum1)
                recip = stat_pool.tile([P, 1], F32, tag="recip")
                nc.vector.reciprocal(out=recip, in_=rsum)

                # O = P V, accumulated over m tiles; P^T via PE transposes.
                o = ps_o.tile([P, 2, 512], F32, tag="o")
                for mi, (m0, msz) in enumerate(M_TILES):
                    tp = ps_t.tile([P, P], F32R, tag="tp")
                    nc.tensor.transpose(
                        tp[:msz, :], p[:, m0 : m0 + msz], ident
                    )
                    pt = pt_pool.tile([P, P], F32R, tag="pt")
                    nc.any.tensor_copy(out=pt[:msz, :], in_=tp[:msz, :])
                    nc.tensor.matmul(
                        o[:, 0, :],
                        pt[:msz, :],
                        v[:msz, mi, 0:512],
                        start=(mi == 0),
                        stop=(mi == 4),
                    )
                    nc.tensor.matmul(
                        o[:, 1, :256],
                        pt[:msz, :],
                        v[:msz, mi, 512:D],
                        start=(mi == 0),
                        stop=(mi == 4),
                    )

                osb = o_pool.tile([P, D], F32, tag="osb")
                nc.scalar.mul(out=osb[:, 0:512], in_=o[:, 0, :], mul=recip)
                nc.scalar.mul(out=osb[:, 512:D], in_=o[:, 1, :256], mul=recip)
                nc.sync.dma_start(out=out[b, nt * P : (nt + 1) * P, :], in_=osb)

    split_multi_waits(nc)
    return nc


_IDENT8 = np.tile(np.eye(P, dtype=np.float32), (N_CORES, 1))

_RUNNER = None
_NC = None


def _bind(hid, img, idn, zout, nc, b_loc):
    operands = [hid, img, idn, zout]
    in_names = ["hidden", "img", "ident", "out"]
    if nc.partition_id_tensor is not None:
        operands.append(bass2jax.partition_id_tensor())
        in_names.append(nc.partition_id_tensor.name)
    return bass2jax._bass_exec_p.bind(
        *operands,
        out_avals=(jax.core.ShapedArray((b_loc, N, D), np.float32),),
        in_names=tuple(in_names),
        out_names=("out",),
        lowering_input_output_aliases=(),
        sim_require_finite=True,
        sim_require_nnan=True,
        nc=nc,
    )


def _make_runner(nc, b_loc: int = B_LOC):
    """Jitted 8-core SPMD executor."""

    def _body(hid, img, idn, zout):
        (o,) = _bind(hid, img, idn, zout, nc, b_loc)
        return (o,)

    mesh = Mesh(np.asarray(jax.devices()[:N_CORES]), ("core",))
    return jax.jit(
        shard_map(
            _body,
            mesh=mesh,
            in_specs=(PartitionSpec("core"),) * 4,
            out_specs=(PartitionSpec("core"),),
            check_rep=False,
        ),
        donate_argnums=(3,),
        keep_unused=True,
    )


def _get_runner():
    global _RUNNER, _NC
    if _RUNNER is None:
        bass2jax.install_neuronx_cc_hook()
        _NC = build_program()
        _RUNNER = _make_runner(_NC, B_LOC)
    return _RUNNER


def kernel(hidden_states: np.ndarray, img_hidden_state: np.ndarray) -> np.ndarray:
    runner = _get_runner()
    (out,) = runner(
        np.ascontiguousarray(hidden_states, dtype=np.float32),
        np.ascontiguousarray(img_hidden_state, dtype=np.float32),
        _IDENT8,
        np.zeros((B, N, D), np.float32),
    )
    return np.asarray(out)


# revision 13
# speedup vs baseline: 2.1086x; 2.1086x over previous
"""Cross-attention fusion kernel for Trainium2 (8 NeuronCores, SPMD).

Computes O = softmax(Q @ K^T) @ V with Q = hidden_states [32,1024,768],
K = V = img_hidden_state [32,576,768], all fp32.

Sharding: data-parallel over batch — 4 batches per core, no collectives.

Per-core dataflow (per batch):
  - V (=K) loaded in natural [m, d] layout; K^T built via PE transposes.
  - Per 128-row Q tile: PE-transpose Q -> Q^T [d, n]; S = Q^T.T @ K^T
    accumulated over d in two PSUM half-banks; softmax along free axis
    (DVE max-reduce, ACT fused exp+row-sum); P^T via PE transposes;
    O = P^T.T @ V accumulated over m; 1/rowsum folded into the
    PSUM->SBUF output copy on ACT.

Matmuls use float32r: fp32 with mantissa rounded to 11 bits, running at
bf16 PE rate (1 cycle/row for free dims >= 256). Inputs are pre-rounded
to the f32r grid on the host so every f32r operand is exact.
"""

from contextlib import ExitStack

import jax
import numpy as np
from jax.experimental.shard_map import shard_map
from jax.sharding import Mesh, PartitionSpec

import concourse.bass as bass
import concourse.tile as tile
from concourse import bass2jax, mybir

F32 = mybir.dt.float32
F32R = mybir.dt.float32r

N_CORES = 8
B, N, M, D = 32, 1024, 576, 768
B_LOC = B // N_CORES  # 4 batches per core
P = 128
NT = N // P  # 8 query tiles per batch
DT = D // P  # 6 contraction tiles
# m tiles: 4 full 128-partition tiles + one 64-row edge tile
M_TILES = [(0, 128), (128, 128), (256, 128), (384, 128), (512, 64)]
MH = 288  # half of M; both S matmul free dims >= 256 keep f32r at full rate


def round_f32r(a: np.ndarray) -> np.ndarray:
    """Round fp32 to the f32r grid (11-bit mantissa, round-half-even)."""
    u = np.ascontiguousarray(a, dtype=np.float32).view(np.uint32)
    low = u & np.uint32(0xFFF)
    base = u & ~np.uint32(0xFFF)
    add = (low > 0x800) | ((low == 0x800) & ((u >> 12) & 1).astype(bool))
    return (base + np.where(add, np.uint32(0x1000), np.uint32(0))).view(np.float32)


def split_multi_waits(nc):
    """Walrus in this toolchain rejects instructions with more than one sync
    wait. Hoist excess waits onto same-engine NoOp carriers placed directly
    before the instruction; engines execute their stream in order, so the
    wait conditions still hold before the instruction issues."""
    carrier_id = 0
    for func in nc.m.functions:
        for bb in func.blocks:
            insts = list(bb.instructions)
            out = []
            changed = False
            for inst in insts:
                si = inst.sync_info
                waits = list(si.on_wait) if si is not None else []
                if len(waits) > 1:
                    changed = True
                    for w in waits[:-1]:
                        nop = mybir.InstNoOp(
                            name=f"waitc-{carrier_id}", engine=inst.engine
                        )
                        carrier_id += 1
                        nop.sync_info = mybir.SyncInfo(on_wait=[w], on_update=[])
                        out.append(nop)
                    inst.sync_info = mybir.SyncInfo(
                        on_wait=waits[-1:], on_update=list(si.on_update)
                    )
                out.append(inst)
            if changed:
                bb.instructions = out


def build_program(b_loc: int = B_LOC, repeat: int = 1):
    nc = bass.Bass("TRN2", target_bir_lowering=False, debug=False)
    hid = nc.dram_tensor("hidden", [b_loc, N, D], F32R, kind="ExternalInput").ap()
    img = nc.dram_tensor("img", [b_loc, M, D], F32R, kind="ExternalInput").ap()
    idn = nc.dram_tensor("ident", [P, P], F32R, kind="ExternalInput").ap()
    out = nc.dram_tensor("out", [b_loc, N, D], F32, kind="ExternalOutput").ap()

    with tile.TileContext(nc) as tc, ExitStack() as ctx:
        rep_cm = tc.For_i(0, repeat, 1) if repeat > 1 else None
        if rep_cm is not None:
            ctx.enter_context(rep_cm)
        const_pool = ctx.enter_context(tc.tile_pool(name="const", bufs=1))
        ident = const_pool.tile([P, P], F32R)
        nc.sync.dma_start(out=ident, in_=idn[:, :])

        kv_pool = ctx.enter_context(tc.tile_pool(name="kv", bufs=2))
        q_pool = ctx.enter_context(tc.tile_pool(name="q", bufs=3))
        qt_pool = ctx.enter_context(tc.tile_pool(name="qt", bufs=3))
        p_pool = ctx.enter_context(tc.tile_pool(name="p", bufs=3))
        pt_pool = ctx.enter_context(tc.tile_pool(name="pt", bufs=3))
        o_pool = ctx.enter_context(tc.tile_pool(name="o", bufs=3))
        stat_pool = ctx.enter_context(tc.tile_pool(name="stat", bufs=4))
        # PSUM: tp 1 bank x4, s0/s1 1 bank x2 each, o 2 banks x1 -> 8 banks
        ps_t = ctx.enter_context(tc.tile_pool(name="ps_t", bufs=4, space="PSUM"))
        ps_s = ctx.enter_context(tc.tile_pool(name="ps_s", bufs=1, space="PSUM"))
        ps_o = ctx.enter_context(tc.tile_pool(name="ps_o", bufs=1, space="PSUM"))

        def transpose_group(dst_ap, src_aps):
            """PE-transpose up to 4 same-shape [pp, ff] blocks into one PSUM
            bank, then move them to SBUF with a single DVE copy."""
            g = len(src_aps)
            pp, ff = src_aps[0].shape
            tp = ps_t.tile([P, 4, P], F32R, tag="tp")
            for gi, s_ap in enumerate(src_aps):
                assert s_ap.shape == (pp, ff)
                nc.tensor.transpose(tp[:ff, gi, :pp], s_ap, ident[:pp, :pp])
            nc.vector.tensor_copy(out=dst_ap, in_=tp[:ff, 0:g, :pp])

        for b in range(b_loc):
            # V in natural [m, d] layout; doubles as the K-transpose source.
            v = kv_pool.tile([P, 5, D], F32R, tag="v")
            for mi, (m0, msz) in enumerate(M_TILES):
                nc.sync.dma_start(out=v[:msz, mi, :], in_=img[b, m0 : m0 + msz, :])
            # K^T [d, m]: six 128-partition d tiles side by side in free dim.
            kT = kv_pool.tile([P, DT, M], F32R, tag="kT")
            for mi, (m0, msz) in enumerate(M_TILES):
                for j0, j1 in ((0, 4), (4, 6)):
                    transpose_group(
                        kT[:, j0:j1, m0 : m0 + msz],
                        [v[:msz, mi, j * P : (j + 1) * P] for j in range(j0, j1)],
                    )

            def stage_front(nt):
                """Load Q tile, build Q^T, S matmuls, softmax -> p/recip."""
                q = q_pool.tile([P, D], F32R, tag="q")
                nc.sync.dma_start(out=q, in_=hid[b, nt * P : (nt + 1) * P, :])
                qT = qt_pool.tile([P, DT, P], F32R, tag="qT")
                for j0, j1 in ((0, 4), (4, 6)):
                    transpose_group(
                        qT[:, j0:j1, :],
                        [q[:, j * P : (j + 1) * P] for j in range(j0, j1)],
                    )

                s0 = ps_s.tile([P, MH], F32, tag="s0")
                s1 = ps_s.tile([P, MH], F32, tag="s1")
                for j in range(DT):
                    qTj = qT[:, j, :]
                    nc.tensor.matmul(
                        s0, qTj, kT[:, j, 0:MH], start=(j == 0), stop=(j == DT - 1)
                    )
                    nc.tensor.matmul(
                        s1, qTj, kT[:, j, MH:M], start=(j == 0), stop=(j == DT - 1)
                    )

                nmax0 = stat_pool.tile([P, 1], F32, tag="nmax0")
                nmax1 = stat_pool.tile([P, 1], F32, tag="nmax1")
                nc.vector.tensor_reduce(
                    out=nmax0, in_=s0, axis=mybir.AxisListType.X,
                    op=mybir.AluOpType.max, negate=True,
                )
                nc.vector.tensor_reduce(
                    out=nmax1, in_=s1, axis=mybir.AxisListType.X,
                    op=mybir.AluOpType.max, negate=True,
                )
                nmax = stat_pool.tile([P, 1], F32, tag="nmax")
                nc.vector.tensor_tensor(
                    out=nmax, in0=nmax0, in1=nmax1, op=mybir.AluOpType.min
                )
                p = p_pool.tile([P, M], F32R, tag="p")
                sum0 = stat_pool.tile([P, 1], F32, tag="sum0")
                sum1 = stat_pool.tile([P, 1], F32, tag="sum1")
                nc.scalar.activation(
                    out=p[:, 0:MH], in_=s0,
                    func=mybir.ActivationFunctionType.Exp,
                    bias=nmax, scale=1.0, accum_out=sum0,
                )
                nc.scalar.activation(
                    out=p[:, MH:M], in_=s1,
                    func=mybir.ActivationFunctionType.Exp,
                    bias=nmax, scale=1.0, accum_out=sum1,
                )
                rsum = stat_pool.tile([P, 1], F32, tag="rsum")
                nc.vector.tensor_add(out=rsum, in0=sum0, in1=sum1)
                recip = stat_pool.tile([P, 1], F32, tag="recip")
                nc.vector.reciprocal(out=recip, in_=rsum)
                return p, recip

            def stage_back(nt, p, recip):
                """P^T, O matmuls, scale, store."""
                pt = pt_pool.tile([P, 5, P], F32R, tag="pt")
                transpose_group(
                    pt[:, 0:4, :], [p[:, m0 : m0 + msz] for m0, msz in M_TILES[:4]]
                )
                transpose_group(pt[:64, 4:5, :], [p[:, 512:M]])
                o = ps_o.tile([P, 2, 512], F32, tag="o")
                for mi, (m0, msz) in enumerate(M_TILES):
                    nc.tensor.matmul(
                        o[:, 0, :], pt[:msz, mi, :], v[:msz, mi, 0:512],
                        start=(mi == 0), stop=(mi == 4),
                    )
                    nc.tensor.matmul(
                        o[:, 1, :256], pt[:msz, mi, :], v[:msz, mi, 512:D],
                        start=(mi == 0), stop=(mi == 4),
                    )
                osb = o_pool.tile([P, D], F32, tag="osb")
                nc.scalar.mul(out=osb[:, 0:512], in_=o[:, 0, :], mul=recip)
                nc.scalar.mul(out=osb[:, 512:D], in_=o[:, 1, :256], mul=recip)
                nc.sync.dma_start(out=out[b, nt * P : (nt + 1) * P, :], in_=osb)

            pending = None
            for nt in range(NT):
                cur = stage_front(nt)
                if pending is not None:
                    stage_back(nt - 1, *pending)
                pending = cur
            stage_back(NT - 1, *pending)

    split_multi_waits(nc)
    return nc


_IDENT8 = np.tile(np.eye(P, dtype=np.float32), (N_CORES, 1))

_RUNNER = None
_NC = None


def _bind(hid, img, idn, zout, nc, b_loc):
    operands = [hid, img, idn, zout]
    in_names = ["hidden", "img", "ident", "out"]
    if nc.partition_id_tensor is not None:
        operands.append(bass2jax.partition_id_tensor())
        in_names.append(nc.partition_id_tensor.name)
    return bass2jax._bass_exec_p.bind(
        *operands,
        out_avals=(jax.core.ShapedArray((b_loc, N, D), np.float32),),
        in_names=tuple(in_names),
        out_names=("out",),
        lowering_input_output_aliases=(),
        sim_require_finite=True,
        sim_require_nnan=True,
        nc=nc,
    )


def _make_runner(nc, b_loc: int = B_LOC):
    """Jitted 8-core SPMD executor."""

    def _body(hid, img, idn, zout):
        (o,) = _bind(hid, img, idn, zout, nc, b_loc)
        return (o,)

    mesh = Mesh(np.asarray(jax.devices()[:N_CORES]), ("core",))
    return jax.jit(
        shard_map(
            _body,
            mesh=mesh,
            in_specs=(PartitionSpec("core"),) * 4,
            out_specs=(PartitionSpec("core"),),
            check_rep=False,
        ),
        donate_argnums=(3,),
        keep_unused=True,
    )


def _get_runner():
    global _RUNNER, _NC
    if _RUNNER is None:
        bass2jax.install_neuronx_cc_hook()
        _NC = build_program()
        _RUNNER = _make_runner(_NC, B_LOC)
    return _RUNNER


def kernel(hidden_states: np.ndarray, img_hidden_state: np.ndarray) -> np.ndarray:
    runner = _get_runner()
    (out,) = runner(
        np.ascontiguousarray(hidden_states, dtype=np.float32),
        np.ascontiguousarray(img_hidden_state, dtype=np.float32),
        _IDENT8,
        np.zeros((B, N, D), np.float32),
    )
    return np.asarray(out)
